# revision 4
# baseline (speedup 1.0000x reference)
"""Trainium2 Bass kernel for nn_KAN_63230508532179 (dense_mlp).

Model (per reference):
  h = gelu(x[:,:,None] * bw1 + bb1)            # [B,1000,16]
  f = tanh(einsum('bnh,noh->bno', h, bw2)+bb2) # [B,1000,8]
  z = f.reshape(B, 8000)
  z = gelu(z @ wc1.T + bc1)                    # [B,256]
  z = gelu(z @ wc2.T + bc2)                    # [B,128]
  y = z @ wc3.T + bc3                          # [B,300]

Strategy: data-parallel over batch across 8 cores (512 rows each); all
on-chip tensors transposed ([feature, batch]).  Branches are processed
in 8-branch groups, 4 groups per pass (A/B half-passes of 2 groups so
PSUM double-buffers within 8 banks).  Layer 1 runs as K=9 row-tiled
matmuls (8 x-rows + ones row per 32-partition strip -> scale and bias
fold into the weights, 4-way tile concurrency).  Layer 2 runs as K=128
M=64 matmuls with 2-way column concurrency; its bias rides the tanh
activation's per-partition bias port.  Combiner-1 accumulates 63
K-chunks into persistent PSUM.  x and w1 ship compact and are expanded
into strip layout by on-chip DMA access patterns.
"""

import os
import sys
from contextlib import ExitStack

sys.path.insert(0, "/opt/trn_rl_repo")
os.environ.setdefault("MYCRO_LOCAL_CACHE", "1")

import numpy as np
import ml_dtypes

import concourse.bass as bass
import concourse.tile as tile
from concourse import bacc, mybir
from concourse.bass_utils import run_bass_kernel_spmd

BF16 = mybir.dt.bfloat16
F32 = mybir.dt.float32
NPBF16 = ml_dtypes.bfloat16

B, N, H1, H2 = 4096, 1000, 16, 8
C1, C2, OUT = 256, 128, 300
NCORES = 8
BC = B // NCORES          # 512 batch rows per core
NG = 128                  # padded groups of 8 branches (125 real)
NP_ = NG * 8              # 1024 padded branches
NPASS = 32                # passes of 4 groups
NCH = 63                  # combiner-1 K-chunks of 128 (2 groups each)

_CACHE = {}


def _build_program():
    if "nc" in _CACHE:
        return _CACHE["nc"]

    nc = bacc.Bacc("TRN2", target_bir_lowering=False, debug=False,
                   num_devices=NCORES)

    # Compact inputs; expanded into strip layout on chip.
    xq_d = nc.dram_tensor("xq", [NPASS * 4 * 9, BC], BF16, kind="ExternalInput")
    w1_d = nc.dram_tensor("w1", [9 * 4 * NPASS, 128], BF16, kind="ExternalInput")
    w2_d = nc.dram_tensor("w2", [128, NG * 64], BF16, kind="ExternalInput")
    b2_d = nc.dram_tensor("b2", [128, 64], F32, kind="ExternalInput")
    wc1_d = nc.dram_tensor("wc1", [128, 64 * 256], BF16, kind="ExternalInput")
    bc1_d = nc.dram_tensor("bc1", [128, 2], F32, kind="ExternalInput")
    wc2_d = nc.dram_tensor("wc2", [128, 256], BF16, kind="ExternalInput")
    bc2_d = nc.dram_tensor("bc2", [128, 1], F32, kind="ExternalInput")
    wc3_d = nc.dram_tensor("wc3", [128, OUT], BF16, kind="ExternalInput")
    bc3_d = nc.dram_tensor("bc3", [128, 3], F32, kind="ExternalInput")
    out_d = nc.dram_tensor("out", [OUT, BC], F32, kind="ExternalOutput")

    AF = mybir.ActivationFunctionType

    with ExitStack() as ctx:
        tc = ctx.enter_context(tile.TileContext(nc))
        consts = ctx.enter_context(tc.tile_pool(name="consts", bufs=1))
        h_pool = ctx.enter_context(tc.tile_pool(name="h", bufs=4))
        f_pool = ctx.enter_context(tc.tile_pool(name="f", bufs=4))
        z_pool = ctx.enter_context(tc.tile_pool(name="z", bufs=1))
        ps_h = ctx.enter_context(tc.tile_pool(name="psh", bufs=2, space="PSUM"))
        ps_f = ctx.enter_context(tc.tile_pool(name="psf", bufs=2, space="PSUM"))
        ps_z = ctx.enter_context(tc.tile_pool(name="psz", bufs=1, space="PSUM"))

        # ---- x: 4 strip DMAs; strip i rows 32i..32i+9 = 8 x rows + ones ----
        xe = consts.tile([128, NPASS * BC], BF16, tag="xe")
        xq_ap = xq_d[:, :].rearrange("(t i r) c -> i r t c", t=NPASS, i=4, r=9)
        for i in range(4):
            nc.sync.dma_start(
                out=xe[32 * i:32 * i + 9, :].rearrange(
                    "r (t c) -> r t c", t=NPASS),
                in_=xq_ap[i])

        # ---- w1: compact [9 rows] per strip, no zero-fill needed ----
        w1s = consts.tile([128, NPASS * 128], BF16, tag="w1s")
        w1_ap = w1_d[:, :].rearrange("(r i p) q -> i r p q", r=9, i=4)
        for i in range(4):
            nc.sync.dma_start(
                out=w1s[32 * i:32 * i + 9, :].rearrange(
                    "r (p q) -> r p q", p=NPASS),
                in_=w1_ap[i])

        # ---- w2 / wc1 / small consts, chunked so the pipeline starts early --
        w2s = consts.tile([128, NG * 64], BF16, tag="w2s")
        for v in range(4):
            nc.sync.dma_start(out=w2s[:, 2048 * v:2048 * (v + 1)],
                              in_=w2_d[:, 2048 * v:2048 * (v + 1)])
        wc1s = consts.tile([128, 64 * 256], BF16, tag="wc1s")
        for v in range(8):
            nc.sync.dma_start(out=wc1s[:, 2048 * v:2048 * (v + 1)],
                              in_=wc1_d[:, 2048 * v:2048 * (v + 1)])

        def load(d, shape, dt, tag):
            s = consts.tile(shape, dt, tag=tag)
            nc.sync.dma_start(out=s[:], in_=d[:, :])
            return s
        b2s = load(b2_d, [128, 64], F32, "b2")
        bc1s = load(bc1_d, [128, 2], F32, "bc1")
        bc2s = load(bc2_d, [128, 1], F32, "bc2")
        wc2s = load(wc2_d, [128, 256], BF16, "wc2")
        wc3s = load(wc3_d, [128, OUT], BF16, "wc3")
        bc3s = load(bc3_d, [128, 3], F32, "bc3")

        # ---- main loop: 63 chunks (2 groups each), half-pass granularity ---
        z1a_ps = ps_z.tile([128, BC], F32, tag="z1a")
        z1b_ps = ps_z.tile([128, BC], F32, tag="z1b")

        for c in range(NCH):
            g0 = 2 * c                      # first group of this chunk
            p = g0 // 4                     # pass index (x/w1 tile column)
            hps = ps_h.tile([128, 2 * BC], F32, tag="hps")
            for j in range(2):
                g = g0 + j
                i = g % 4                   # strip
                nc.tensor.matmul(
                    hps[:, BC * j:BC * (j + 1)],
                    lhsT=w1s[32 * i:32 * i + 9, 128 * p:128 * (p + 1)],
                    rhs=xe[32 * i:32 * i + 9, BC * p:BC * (p + 1)],
                    start=True, stop=True, tile_position=(32 * i, 0))
            hsb = h_pool.tile([128, 2 * BC], BF16, tag="hsb")
            nc.scalar.activation(hsb[:], hps[:], AF.Gelu)
            fps = ps_f.tile([128, BC], F32, tag="fps")
            for j in range(2):
                g = g0 + j
                nc.tensor.matmul(
                    fps[64 * j:64 * (j + 1), :],
                    lhsT=w2s[:, 64 * g:64 * (g + 1)],
                    rhs=hsb[:, BC * j:BC * (j + 1)],
                    start=True, stop=True, tile_position=(0, 64 * j))
            fsb = f_pool.tile([128, BC], BF16, tag="fsb")
            nc.scalar.activation(fsb[:], fps[:], AF.Tanh,
                                 bias=b2s[:, c:c + 1], scale=1.0)
            last = c == NCH - 1
            nc.tensor.matmul(z1a_ps[:], lhsT=wc1s[:, 256 * c:256 * c + 128],
                             rhs=fsb[:], start=(c == 0), stop=last,
                             skip_group_check=True)
            nc.tensor.matmul(z1b_ps[:], lhsT=wc1s[:, 256 * c + 128:256 * c + 256],
                             rhs=fsb[:], start=(c == 0), stop=last,
                             skip_group_check=True)

        # ---- combiner tail ----
        z1a = z_pool.tile([128, BC], BF16, tag="z1a_sb")
        z1b = z_pool.tile([128, BC], BF16, tag="z1b_sb")
        nc.scalar.activation(z1a[:], z1a_ps[:], AF.Gelu,
                             bias=bc1s[:, 0:1], scale=1.0)
        nc.scalar.activation(z1b[:], z1b_ps[:], AF.Gelu,
                             bias=bc1s[:, 1:2], scale=1.0)

        z2_ps = ps_h.tile([128, BC], F32, tag="hps")
        nc.tensor.matmul(z2_ps[:], lhsT=wc2s[:, 0:128], rhs=z1a[:],
                         start=True, stop=False, skip_group_check=True)
        nc.tensor.matmul(z2_ps[:], lhsT=wc2s[:, 128:256], rhs=z1b[:],
                         start=False, stop=True, skip_group_check=True)
        z2 = z_pool.tile([128, BC], BF16, tag="z2_sb")
        nc.scalar.activation(z2[:], z2_ps[:], AF.Gelu,
                             bias=bc2s[:, 0:1], scale=1.0)

        for i, m in ((0, 128), (1, 128), (2, 44)):
            o_ps = ps_f.tile([128, BC], F32, tag="fps")
            nc.tensor.matmul(o_ps[0:m, :], lhsT=wc3s[:, 128 * i:128 * i + m],
                             rhs=z2[:], start=True, stop=True)
            o_sb = z_pool.tile([128, BC], F32, tag=f"o{i}")
            nc.vector.tensor_scalar_add(o_sb[0:m, :], o_ps[0:m, :],
                                        bc3s[0:m, i:i + 1])
            nc.sync.dma_start(out=out_d[128 * i:128 * i + m, :],
                              in_=o_sb[0:m, :])

    nc.compile()
    _CACHE["nc"] = nc
    return nc


def preprocess(x, bw1, bb1, bw2, bb2, wc1, bc1, wc2, bc2, wc3, bc3):
    """Host-side repack of full inputs into per-core input maps."""
    f32 = np.float32
    bw1p = np.zeros((NP_, H1), f32); bw1p[:N] = bw1
    bb1p = np.zeros((NP_, H1), f32); bb1p[:N] = bb1
    bw2p = np.zeros((NP_, H2, H1), f32); bw2p[:N] = bw2
    bb2p = np.zeros((NP_, H2), f32); bb2p[:N] = bb2

    # x compact: xq[t, i, r, c]; r<8 -> x.T[8*(4t+i)+r], r=8 -> 1.0
    xr = np.zeros((NP_, B), f32)
    xr[:N] = x.T
    xq = np.ones((NPASS, 4, 9, B), f32)
    xq[:, :, :8, :] = xr.reshape(NPASS, 4, 8, B)
    xq = xq.reshape(NPASS * 4 * 9, B).astype(NPBF16)

    # w1 compact: w1c[r, i, p, q], q = 16*bb + k
    # r == bb -> bw1[8*(4p+i)+bb, k];  r == 8 -> bb1[...]
    W1 = np.zeros((9, 4, NPASS, 128), f32)
    bw1g = bw1p.reshape(NPASS, 4, 8, H1)     # [p, i, bb, k]
    bb1g = bb1p.reshape(NPASS, 4, 8, H1)
    for bb in range(8):
        W1[bb, :, :, 16 * bb:16 * bb + 16] = \
            bw1g[:, :, bb, :].transpose(1, 0, 2)
        W1[8, :, :, 16 * bb:16 * bb + 16] = \
            bb1g[:, :, bb, :].transpose(1, 0, 2)
    w1_sb = W1.reshape(9 * 4 * NPASS, 128).astype(NPBF16)

    # w2 block-diagonal per group: [128=(bb,k), 64=(bb,o)]
    W2 = np.zeros((NG, 128, 64), f32)
    bw2g = bw2p.reshape(NG, 8, H2, H1)       # [g, bb, o, k]
    for bb in range(8):
        W2[:, 16 * bb:16 * (bb + 1), 8 * bb:8 * (bb + 1)] = \
            bw2g[:, bb].transpose(0, 2, 1)   # [g, k, o]
    w2_sb = W2.transpose(1, 0, 2).reshape(128, NG * 64).astype(NPBF16)
    b2_sb = np.ascontiguousarray(bb2p.reshape(64, 128).T)

    # combiner 1: wc1 [256, 8000] -> chunk-major transposed tiles (64 chunks)
    wc1p = np.zeros((C1, NP_ * H2), f32)
    wc1p[:, :N * H2] = wc1
    wc1_sb = np.ascontiguousarray(
        wc1p.T.reshape(64, 128, C1).transpose(1, 0, 2).reshape(128, 64 * C1)
    ).astype(NPBF16)
    bc1_sb = np.ascontiguousarray(bc1.reshape(2, 128).T.astype(f32))

    wc2_sb = np.ascontiguousarray(
        wc2.T.reshape(2, 128, C2).transpose(1, 0, 2).reshape(128, 256)
    ).astype(NPBF16)
    bc2_sb = np.ascontiguousarray(bc2.reshape(C2, 1).astype(f32))

    wc3_sb = np.ascontiguousarray(wc3.T).astype(NPBF16)   # [128, 300]
    bc3p = np.zeros(384, f32); bc3p[:OUT] = bc3
    bc3_sb = np.ascontiguousarray(bc3p.reshape(3, 128).T)

    shared = {
        "w1": w1_sb, "w2": w2_sb, "b2": b2_sb,
        "wc1": wc1_sb, "bc1": bc1_sb, "wc2": wc2_sb, "bc2": bc2_sb,
        "wc3": wc3_sb, "bc3": bc3_sb,
    }
    in_maps = []
    for c in range(NCORES):
        m = dict(shared)
        m["xq"] = np.ascontiguousarray(xq[:, BC * c:BC * (c + 1)])
        in_maps.append(m)
    return in_maps


def run(in_maps, trace=False):
    nc = _build_program()
    return run_bass_kernel_spmd(nc, in_maps, list(range(NCORES)), trace=trace)


def kernel(x, bw1, bb1, bw2, bb2, wc1, bc1, wc2, bc2, wc3, bc3):
    args = [np.asarray(a, np.float32) for a in
            (x, bw1, bb1, bw2, bb2, wc1, bc1, wc2, bc2, wc3, bc3)]
    in_maps = preprocess(*args)
    res = run(in_maps, trace=False)
    y = np.empty((B, OUT), np.float32)
    for c in range(NCORES):
        y[BC * c:BC * (c + 1), :] = res.results[c]["out"].T
    return y
